# revision 56
# baseline (speedup 1.0000x reference)
"""NT-Xent loss kernel for Trainium2, distributed across 8 NeuronCores.

Strategy: each core receives the full [8192, 128] input, rotated by 1024*c
rows so the kernel is pure SPMD — every core computes the row sums of
exp(sim/T) for the *first* 1024 local rows against all 8192 columns.

Per core:
  - load x (rows-on-partitions layout) via Pool SWDGE (earliest queue start)
  - norms:  s = sum(x^2, axis=d)  (DVE square -> bf16, DVE segmented reduce)
  - r = s^(-1/2) via exp(-0.5 * ln(s))  (keeps ACT in the exp/ln table set)
  - normalize rows -> bf16 (DVE tensor_scalar, r staged DVE-local)
  - pair 0 transposes through a tiny 1-buf prep psum pool released right
    after its DVE copy, so the main psum pool's alloc boundary waits only
    on copy-0 (a pool allocated over released banks waits the release)
  - pairs 1-3 transpose THROUGH the main pm ring mid-stream via
    byte-compatible [128,4096]bf16 slot views; an "ident patch" (DVE bypass
    op rewriting ident[0,0] while reading exp#(8g-1)'s output) makes the
    transposes' single DVE wait imply the psum slot releases
  - main loop: 8 Mtiles x 4 col-groups: 4 bf16 matmuls -> PSUM [128,2048],
    one ACT pass exp(2*sim) with accum_out giving partial row sums
  - row totals - e^2 (diag), ln + accumulate, minus 2*sum(pos-pair sims),
    partition-reduce via ones-matmul -> scalar partial loss
Host sums the 8 partial scalars.

Hardware constraint honored throughout: a Matmult instruction can carry
only ONE sync-wait, so every matmul's dependencies (operands + PSUM slot
release) are arranged to come from a single semaphore. The strip pass
makes per-instruction choices keyed by the instruction NAMES captured at
emission (the instruction stream it walks is in SCHEDULED order, so
positional counting would mis-classify reordered matmuls).
"""

import numpy as np

import concourse.bass as bass
import concourse.tile as tile
from concourse import mybir
from concourse.bass_utils import run_bass_kernel_spmd
from concourse.masks import make_identity

N2 = 8192          # total rows (2N)
D = 128            # feature dim
NCORES = 8
RPC = N2 // NCORES  # rows per core = 1024
NCHUNK = N2 // 128  # 64 chunks of 128 rows
F32 = mybir.dt.float32
BF16 = mybir.dt.bfloat16
AF = mybir.ActivationFunctionType
ALU = mybir.AluOpType
E2 = float(np.exp(2.0, dtype=np.float64))  # diag term exp(sim_ii / T), T=0.5


def _emit(tc: tile.TileContext, ctx, out_ap: bass.AP, x_ap: bass.AP):
    nc = tc.nc
    # name -> "dve" | "act": which single wait each main matmul must keep
    mm_keep: dict[str, str] = {}

    big = ctx.enter_context(tc.tile_pool(name="big", bufs=1))
    esc = ctx.enter_context(tc.tile_pool(name="esc", bufs=3))
    small = ctx.enter_context(tc.tile_pool(name="small", bufs=1))

    x_g = [
        big.tile([128, 8, 128], F32, tag=f"x{g}", name=f"x_{g}") for g in range(8)
    ]
    xsq_g = [
        big.tile([128, 8, 128], BF16, tag=f"xsq{g}", name=f"xsq_{g}")
        for g in range(8)
    ]
    xb = big.tile([128, NCHUNK, 128], BF16, tag="xb")     # normalized, bf16
    xnT = [
        big.tile([128, 2048], BF16, tag=f"xnT{t}", name=f"xnT_{t}")
        for t in range(4)
    ]

    s = small.tile([128, NCHUNK], BF16)    # squared norms (row 128c+p at [p, c])
    ls = small.tile([128, NCHUNK], F32)
    r = small.tile([128, NCHUNK], F32)     # 1/norm
    r_dve = small.tile([128, NCHUNK], F32)  # DVE-local copy (TS 1-wait rule)
    iprobe = small.tile([1, 1], BF16)      # DVE probe of ident (Pool->DVE edge)
    rs = small.tile([128, 32], F32)        # accum slots (m, g)
    rt = small.tile([128, 8], F32)         # row totals per Mtile
    lg = small.tile([128, 8], F32)
    logsum = small.tile([128, 1], F32)
    possum = small.tile([128, 1], F32)
    fin = small.tile([128, 1], F32)
    fin2 = small.tile([128, 1], F32)       # ACT-written copy (matmul 1-wait rule)
    ones = small.tile([128, 1], F32)       # ACT-written
    ident = small.tile([128, 128], BF16)
    fin_sb = small.tile([1, 1], F32)
    pos_scr = small.tile([128, RPC], BF16)
    negE2 = small.tile([128, 1], F32)

    nc.vector.memset(negE2, -E2)
    make_identity(nc, ident)
    # DVE probe-read of ident: every later DVE op now transitively implies
    # the identity is built, letting the strip pass drop Pool waits from
    # the transpose matmuls (which can carry only one sync wait).
    nc.vector.tensor_copy(iprobe, ident[0:1, 0:1])
    # ones written by ACT so the final matmul waits on ACT only
    nc.scalar.activation(out=ones, in_=negE2, func=AF.Copy, bias=1.0, scale=0.0)

    x_src = x_ap.rearrange("(c p) d -> p c d", p=128)

    # input loads on the Pool SWDGE queues
    for g in range(8):
        sl = slice(8 * g, 8 * g + 8)
        nc.gpsimd.dma_start(out=x_g[g][:, :, :], in_=x_src[:, sl, :])

    def prep_group(g):
        sl = slice(8 * g, 8 * g + 8)
        nc.vector.tensor_mul(
            xsq_g[g][:, :, :], x_g[g][:, :, :], x_g[g][:, :, :]
        )
        # bf16 sum of 128 squares: rel err ~0.4%/sqrt(128), fine at 2e-2
        with nc.allow_low_precision(reason="norms tolerate bf16"):
            nc.vector.tensor_reduce(
                out=s[:, sl],
                in_=xsq_g[g][:, :, :],
                axis=mybir.AxisListType.X,
                op=ALU.add,
            )
        # r = exp(-0.5*ln(s)) == s^-1/2 ; exp+ln share one ACT table set
        nc.scalar.activation(out=ls[:, sl], in_=s[:, sl], func=AF.Ln)
        nc.scalar.activation(
            out=r[:, sl], in_=ls[:, sl], func=AF.Exp, scale=-0.5
        )
        nc.vector.tensor_copy(r_dve[:, sl], r[:, sl])
        for c in range(8 * g, 8 * g + 8):
            nc.vector.tensor_scalar_mul(
                out=xb[:, c, :],
                in0=x_g[c // 8][:, c % 8, :],
                scalar1=r_dve[:, c : c + 1],
            )

    def transpose_pair(tg, pt):
        for k in range(16):
            ch = 16 * tg + k
            nc.tensor.transpose(
                pt[:, 128 * k : 128 * (k + 1)], xb[:, ch, :], ident
            )
        # copy on DVE (off the ACT exp stream). Group tg's first two slab
        # matmuls wait DVE >= this copy; because the copy's queue position
        # is necessarily after the ident patch (in-order DVE, patch gates
        # the transposes feeding it), that single DVE wait also implies
        # the psum slot releases exp#(8tg-2)/exp#(8tg-1).
        nc.vector.tensor_copy(xnT[tg][:, :], pt[:, :])

    # ---- pair 0 through a tiny 1-buf prep pool, released right after its
    # copy: the main pool's alloc boundary then waits only on copy-0.
    with tc.tile_pool(name="prep_ps", bufs=1, space="PSUM") as prep_ps:
        prep_group(0)
        prep_group(1)
        pt0 = prep_ps.tile([128, 2048], BF16, tag="pt", name="pt_0")
        transpose_pair(0, pt0)

    # ---- main loop: pairs 1-3 transpose THROUGH the main pm ring
    # mid-stream. All remaining norms/normalize emitted first: the in-order
    # DVE queue must not have TS work sitting behind an ident patch (which
    # waits on a main-loop exp).
    for gg in range(2, 8):
        prep_group(gg)
    ps = ctx.enter_context(tc.tile_pool(name="ps", bufs=2, space="PSUM"))
    e_tiles = []
    for g in range(4):
        if g >= 1:
            # ident patch: rewrite ident[0,0] with its own value (bypass op)
            # while READING exp#(8g-1)'s output — past the exps that release
            # the pm slots taken by this pair's pt view AND the group's
            # first two slabs. Every transpose reads ident, so each one's
            # DVE data wait lands at/after this patch.
            # (e*0)+1 = exactly 1.0: the write value must not depend on the
            # e read at all — the e_t slot may be WAR-raced by a later exp
            # (we strip that wait), and the HW ALU's "bypass" operand order
            # is not trusted
            rel = e_tiles[8 * g - 1]
            nc.vector.tensor_scalar(
                out=ident[0:1, 0:1],
                in0=rel[0:1, 0:1],
                scalar1=0.0,
                scalar2=1.0,
                op0=ALU.mult,
                op1=ALU.add,
            )
            ptg = ps.tile([128, 4096], BF16, tag="pm", name=f"pt_{g}")
            transpose_pair(g, ptg[:, 0:2048])
        for m in range(8):
            pm = ps.tile([128, 2048], F32, tag="pm", name=f"pm_{m}_{g}")
            lhsT = xnT[0][:, 128 * m : 128 * (m + 1)]
            for k in range(4):
                mm = nc.tensor.matmul(
                    pm[:, 512 * k : 512 * (k + 1)],
                    lhsT=lhsT,
                    rhs=xnT[g][:, 512 * k : 512 * (k + 1)],
                    start=True,
                    stop=True,
                )
                # slabs m<2: single DVE wait (the xnT copy, whose in-order
                # DVE position implies the ident patch -> slot releases).
                # slabs m>=2: single ACT wait (their slot exp, which is
                # downstream of the group's m=0 matmuls -> implies the copy).
                mm_keep[mm.ins.name] = "dve" if m < 2 else "act"
            e_t = esc.tile([128, 2048], BF16, tag="e", name=f"e_{m}_{g}")
            e_tiles.append(e_t)
            j = 8 * g + m
            nc.scalar.activation(
                out=e_t[:, :],
                in_=pm[:, :],
                func=AF.Exp,
                scale=2.0,
                accum_out=rs[:, j : j + 1],
            )

    # ---- positive-pair term: sum over my rows of sim(i, i+N) ----
    nc.vector.tensor_mul(pos_scr, xnT[0][:, 0:RPC], xnT[2][:, 0:RPC])
    nc.vector.tensor_reduce(
        out=possum, in_=pos_scr, axis=mybir.AxisListType.X, op=ALU.add
    )

    # ---- finals ----
    nc.vector.tensor_reduce(
        out=rt,
        in_=rs.rearrange("p (g m) -> p m g", m=8),
        axis=mybir.AxisListType.X,
        op=ALU.add,
    )
    nc.scalar.activation(
        out=lg, in_=rt, func=AF.Ln, bias=negE2[:, :], scale=1.0, accum_out=logsum
    )
    nc.vector.scalar_tensor_tensor(
        out=fin,
        in0=possum,
        scalar=-2.0,
        in1=logsum,
        op0=ALU.mult,
        op1=ALU.add,
    )
    nc.scalar.copy(fin2, fin)  # ACT hop: final matmul waits on ACT only
    pf = ps.tile([128, 2048], F32, tag="pm", name="pf")
    mm = nc.tensor.matmul(
        pf[0:1, 0:1].bitcast(F32), lhsT=fin2, rhs=ones, start=True, stop=True
    )
    mm_keep[mm.ins.name] = "act"
    nc.vector.tensor_copy(fin_sb, pf[0:1, 0:1])
    # SWDGE for the tiny output write: the HWDGE direct-2D encoding only
    # carries one sync wait and this DMA needs a data wait on DVE
    nc.gpsimd.dma_start(out=out_ap, in_=fin_sb)
    return mm_keep


def _strip_self_waits(nc, mm_keep):
    """Reduce every wait list to what the single-wait encodings allow.

    PE and ACT are strict in-order single queues whose semaphores increment
    at instruction completion in program order, so engine-self waits are
    always implied. Main matmuls keep exactly the wait recorded for them at
    emission (by NAME — the post-schedule instruction order is not the
    emission order). Transposes keep DVE only (normalize + ident patch;
    ident's Pool wait is implied by the initial iprobe). ACT instructions
    that carry {PE, DVE}: the DVE entry is a WAR of an ident-patch READ
    whose value is discarded (bypass op) — numerically benign, dropped.
    """
    eng_prefix = {
        mybir.EngineType.PE: "PE_",
        mybir.EngineType.Activation: "Activation_",
        mybir.EngineType.DVE: "DVE_",
        mybir.EngineType.Pool: "Pool_",
    }
    for bb in nc.main_func.blocks:
        for ins in bb.instructions:
            si = ins.sync_info
            if si is None:
                continue
            tn = type(ins).__name__
            if tn == "InstDrain":
                w = list(si.on_wait)
                if len(w) > 1 and any(
                    (x.ant_name or "").startswith("DMASW0") for x in w
                ):
                    # keep only the out-DMA's queue sem: the x-load queues
                    # completed before their consumers (squares), which are
                    # upstream of the output value this queue's DMA carries
                    si.on_wait = [
                        x for x in w if (x.ant_name or "").startswith("DMASW0")
                    ]
                continue
            if tn == "InstDMACopy":
                # the output DMA: its SWDGE-queue wait (x loads drained) is
                # implied by the DVE data wait — fin_sb is downstream of
                # every byte of x
                w = list(si.on_wait)
                if len(w) > 1 and any(
                    (x.ant_name or "").startswith("DVE_") for x in w
                ):
                    si.on_wait = [
                        x for x in w if (x.ant_name or "").startswith("DVE_")
                    ]
                continue
            if tn != "InstMatmult":
                pfx = eng_prefix.get(getattr(ins, "engine", None))
                if pfx is None:
                    continue
                w = list(si.on_wait)
                w2 = [x for x in w if not (x.ant_name or "").startswith(pfx)]
                if tn in ("InstTensorTensor", "InstTensorScalarPtr") and any(
                    (x.ant_name or "").startswith("Activation_") for x in w2
                ):
                    # ident patches: their PE wait (WAR vs the previous
                    # pair's transposes) is implied by the ACT exp wait
                    w2 = [
                        x for x in w2 if not (x.ant_name or "").startswith("PE_")
                    ]
                if tn == "InstActivation" and any(
                    (x.ant_name or "").startswith("PE_") for x in w2
                ):
                    # exps whose e_t slot was READ by an ident patch: the
                    # WAR'd value is discarded by the patch (bypass op)
                    w2 = [
                        x for x in w2 if not (x.ant_name or "").startswith("DVE_")
                    ]
                if len(w2) != len(w):
                    si.on_wait = w2
                continue
            # Matmult: strip to the single allowed wait
            w = list(si.on_wait)
            w2 = [x for x in w if not (x.ant_name or "").startswith("PE_")]
            if getattr(ins, "is_transpose", False):
                w2 = [x for x in w2 if (x.ant_name or "").startswith("DVE_")]
            else:
                keep = mm_keep.get(ins.name)
                assert keep is not None, f"unclassified matmul {ins.name}"
                pfx = "DVE_" if keep == "dve" else "Activation_"
                kept = [x for x in w2 if (x.ant_name or "").startswith(pfx)]
                if not kept:
                    # fall back to whatever single-engine wait exists rather
                    # than silently dropping all ordering
                    kept = w2[:1]
                w2 = kept
            si.on_wait = w2


def _build(strip: bool = True):
    from contextlib import ExitStack

    nc = bass.Bass("TRN2", debug=False, num_devices=NCORES)
    x_in = nc.dram_tensor("x", [N2, D], F32, kind="ExternalInput")
    out = nc.dram_tensor("out", [1, 1], F32, kind="ExternalOutput")
    with tile.TileContext(nc) as tc:
        with ExitStack() as ctx:
            mm_keep = _emit(tc, ctx, out.ap(), x_in.ap())
    if strip:
        _strip_self_waits(nc, mm_keep)
    return nc


_NC_CACHE = None


def _get_nc():
    global _NC_CACHE
    if _NC_CACHE is None:
        _NC_CACHE = _build()
    return _NC_CACHE


def kernel(**inputs) -> np.ndarray:
    x = np.ascontiguousarray(
        np.asarray(inputs["projected_vectors"]), dtype=np.float32
    )
    assert x.shape == (N2, D)
    nc = _get_nc()
    in_maps = [
        {"x": np.ascontiguousarray(np.roll(x, -RPC * c, axis=0))}
        for c in range(NCORES)
    ]
    res = run_bass_kernel_spmd(nc, in_maps, core_ids=list(range(NCORES)))
    total = np.float32(0.0)
    for rmap in res.results:
        total += np.float32(rmap["out"][0, 0])
    return np.asarray(total, dtype=np.float32)


if __name__ == "__main__":
    xt = np.random.randn(N2, D).astype(np.float32)
    print(kernel(projected_vectors=xt))
